# revision 3
# baseline (speedup 1.0000x reference)
"""MultiHeadLatentAttention on 8 Trainium2 NeuronCores (Bass/Tile, SPMD).

Sharding (tensor parallel over heads, per the hint, plus two refinements):
  - 16 heads / 8 cores = 2 heads per core: q_proj + kv_b_proj output dims and
    o_proj input dim sharded by head.
  - kv_a_proj + rms-norm are token-sharded (512 tokens/core) with an
    AllGather of the normalized latent (small: 1 MB/core) instead of
    replicating the 9.7 GFLOP kv_a matmul on every core.
  - Instead of an AllReduce of full [B,S,H] partial o_proj outputs (33 MB,
    ~380 us), an AllToAll of the attention outputs (4 MB) token-shards the
    o_proj: each core computes the full o_proj for 512 tokens and outputs
    exactly its token slice. Host-side unshard is a pure concat/transpose.

All matmuls run as fp32r (TF32: 10-bit mantissa inputs, fp32 accumulate) at
1 cycle/row on the PE. Inputs are pre-rounded to the TF32 grid on the host;
intermediates are rounded by the producing engine writing float32r.

Layouts keep tokens on the moving/free axis everywhere:
  hsT [hid, tok], qT/kT/vT [d, tok] per (head, batch), scoresT [ktok, qtok]
  (softmax along partitions via ones-matmul), attention out [d, tok],
  o_proj out [out, tok] (host transposes at the end).
"""

import math
from contextlib import ExitStack

import numpy as np

B, S = 2, 2048
T = B * S                     # 4096 flattened tokens
HID = 2048
H, D = 16, 128
RANK, ROPE = 512, 64
MAX_POS, ORIG_POS = 131072, 8192
BASE = 500000.0
BETA_FAST, BETA_SLOW = 32.0, 1.0
EPS = 1e-6
NCORES = 8
HPC = H // NCORES             # 2 heads per core
TPC = T // NCORES             # 512 tokens per core (kv_a shard)
SPC = S // NCORES             # 256 tokens per (core, batch) after AllToAll

_CACHE: dict = {}


def tf32_round(x: np.ndarray) -> np.ndarray:
    u = np.ascontiguousarray(x, dtype=np.float32).view(np.uint32).copy()
    add = ((u >> 13) & 1) + 0xFFF
    u = (u + add) & np.uint32(0xFFFFE000)
    return u.view(np.float32)


def _yarn_cos_sin():
    """cos/sin tables matching reference.py's yarn_cos_sin (mscale folded)."""
    scaling = MAX_POS / ORIG_POS
    pos_freqs = BASE ** (np.arange(0, ROPE, 2, dtype=np.float64) / ROPE)
    extrap = 1.0 / pos_freqs
    interp = 1.0 / (scaling * pos_freqs)
    low = max(math.floor(ROPE * math.log(ORIG_POS / (BETA_FAST * 2 * math.pi))
                         / (2 * math.log(BASE))), 0)
    high = min(math.ceil(ROPE * math.log(ORIG_POS / (BETA_SLOW * 2 * math.pi))
                         / (2 * math.log(BASE))), ROPE - 1)
    i = np.arange(ROPE // 2, dtype=np.float64)
    smooth = np.clip((i - low) / max(high - low, 1), 0.0, 1.0)
    inv_freq = ((1.0 - smooth) * interp + smooth * extrap).astype(np.float32)
    pos = np.arange(S, dtype=np.float32)
    freqs = pos[:, None] * inv_freq[None, :]              # [S, 32]
    emb = np.concatenate([freqs, freqs], axis=-1)         # [S, 64]
    mscale = 0.1 * math.log(scaling) + 1.0
    cos = (np.cos(emb) * mscale).astype(np.float32)
    sin = (np.sin(emb) * mscale).astype(np.float32)
    return cos.T.copy(), sin.T.copy()                     # [64, S] each


def build_nc():
    """Build + compile the (single, SPMD) Bass program for all 8 cores."""
    import concourse.tile as tile
    import concourse.mybir as mybir
    from concourse import bacc

    F32 = mybir.dt.float32
    F32R = mybir.dt.float32r
    AF = mybir.ActivationFunctionType
    RG = [list(range(NCORES))]

    nc = bacc.Bacc("TRN2", target_bir_lowering=False, debug=False,
                   num_devices=NCORES)

    # ---- kernel I/O ----
    hsT_in = nc.dram_tensor("hsT", [HID, T], F32R, kind="ExternalInput").ap()
    hsmy_in = nc.dram_tensor("hsmy", [HID, TPC], F32R, kind="ExternalInput").ap()
    qwT_in = nc.dram_tensor("qwT", [HID, HPC * D], F32R, kind="ExternalInput").ap()
    kvaT_in = nc.dram_tensor("kvaT", [HID, RANK], F32R, kind="ExternalInput").ap()
    kvbT_in = nc.dram_tensor("kvbT", [RANK, HPC * 2 * D], F32R, kind="ExternalInput").ap()
    owt_in = nc.dram_tensor("owt", [16, 128, HID], F32R, kind="ExternalInput").ap()
    cos_in = nc.dram_tensor("cos", [ROPE, S], F32, kind="ExternalInput").ap()
    sinsh_in = nc.dram_tensor("sinsh", [ROPE, S], F32, kind="ExternalInput").ap()
    ident_in = nc.dram_tensor("ident", [128, 128], F32R, kind="ExternalInput").ap()
    ones_in = nc.dram_tensor("ones", [128, 128], F32R, kind="ExternalInput").ap()
    outT = nc.dram_tensor("outT", [HID, 2 * SPC], F32, kind="ExternalOutput").ap()

    NH = HID // 128   # 16 hid chunks
    NR = RANK // 128  # 4 rank chunks

    with tile.TileContext(nc) as tc, ExitStack() as ctx0:
        const = ctx0.enter_context(tc.tile_pool(name="const", bufs=1))
        dram = ctx0.enter_context(tc.tile_pool(name="dram", bufs=1, space="DRAM"))

        ident = const.tile([128, 128], F32R)
        ones = const.tile([128, 128], F32R)
        cosb = const.tile([ROPE, S], F32)
        sinsh = const.tile([ROPE, S], F32)
        eps_t = const.tile([1, 1], F32)
        nc.sync.dma_start(ident[:], ident_in[:])
        nc.sync.dma_start(ones[:], ones_in[:])
        nc.sync.dma_start(cosb[:], cos_in[:])
        nc.sync.dma_start(sinsh[:], sinsh_in[:])
        nc.vector.memset(eps_t[:], EPS)

        # collective bounce buffers
        ag_in = dram.tile([TPC, RANK], F32R)
        ag_out = dram.tile([NCORES, TPC, RANK], F32R, addr_space="Shared")
        a2a_in = [dram.tile([NCORES, HPC * D, SPC], F32R, name=f"a2ain{b}")
                  for b in range(B)]
        a2a_out = [dram.tile([NCORES, HPC * D, SPC], F32R, name=f"a2aout{b}")
                   for b in range(B)]

        with ExitStack() as ctx_big:
            big = ctx_big.enter_context(tc.tile_pool(name="big", bufs=1))
            # per (head j, batch b) tiles, [128, S] each
            qT = [[big.tile([128, S], F32R, name=f"qT{j}{b}") for b in range(B)]
                  for j in range(HPC)]
            kT = [[big.tile([128, S], F32R, name=f"kT{j}{b}") for b in range(B)]
                  for j in range(HPC)]
            vnat = [[big.tile([128, S], F32R, name=f"vn{j}{b}") for b in range(B)]
                    for j in range(HPC)]
            aoT = [[big.tile([128, S], F32R, name=f"ao{j}{b}") for b in range(B)]
                   for j in range(HPC)]

            # ---------- P1: kv_a on my 512-token shard + rms norm + AllGather
            with ExitStack() as c1:
                p1 = c1.enter_context(tc.tile_pool(name="p1", bufs=1))
                p1ps = c1.enter_context(tc.tile_pool(name="p1ps", bufs=1, space="PSUM"))
                ps_lat = [p1ps.tile([128, TPC], F32, name=f"pslat{m}", tag=f"lat{m}")
                          for m in range(NR)]
                for k in range(NH):
                    kva_t = p1.tile([128, RANK], F32R, tag="kvat", bufs=3)
                    nc.sync.dma_start(kva_t[:], kvaT_in[k * 128:(k + 1) * 128, :])
                    ht = p1.tile([128, TPC], F32R, tag="hsmy", bufs=4)
                    nc.sync.dma_start(ht[:], hsmy_in[k * 128:(k + 1) * 128, :])
                    for m in range(NR):
                        nc.tensor.matmul(
                            ps_lat[m][:],
                            kva_t[:, m * 128:(m + 1) * 128],
                            ht[:], start=(k == 0), stop=(k == NH - 1))
                # rms norm over rank (partition axis, 4 chunks)
                lat_sb = p1.tile([128, NR * TPC], F32)
                ps_var = p1ps.tile([1, TPC], F32, tag="var")
                for m in range(NR):
                    nc.any.tensor_copy(lat_sb[:, m * TPC:(m + 1) * TPC], ps_lat[m][:])
                sq = [p1.tile([128, TPC], F32, name=f"sq{m}", tag="sq", bufs=2)
                      for m in range(NR)]
                for m in range(NR):
                    nc.vector.tensor_mul(sq[m][:], lat_sb[:, m * TPC:(m + 1) * TPC],
                                         lat_sb[:, m * TPC:(m + 1) * TPC])
                    nc.tensor.matmul(ps_var[:], ones[:, 0:1].bitcast(F32), sq[m][:],
                                     start=(m == 0), stop=(m == NR - 1))
                std = p1.tile([1, TPC], F32, tag="std")
                nc.scalar.activation(std[:], ps_var[:], AF.Sqrt,
                                     bias=eps_t[:], scale=1.0 / RANK)
                istd = p1.tile([1, TPC], F32, tag="istd")
                nc.vector.reciprocal(istd[:], std[:])
                ps_bc = p1ps.tile([128, TPC], F32, tag="bc")
                nc.tensor.matmul(ps_bc[:], ones[0:1, :].bitcast(F32), istd[:],
                                 start=True, stop=True)
                latn = p1.tile([128, NR * TPC], F32R)
                for m in range(NR):
                    nc.vector.tensor_mul(latn[:, m * TPC:(m + 1) * TPC],
                                         lat_sb[:, m * TPC:(m + 1) * TPC], ps_bc[:])
                    nc.sync.dma_start(ag_in[m * 128:(m + 1) * 128, :],
                                      latn[:, m * TPC:(m + 1) * TPC])
                nc.gpsimd.collective_compute(
                    "AllGather", mybir.AluOpType.bypass, replica_groups=RG,
                    ins=[ag_in.opt()], outs=[ag_out.opt()])

            # ---------- P2: q_proj for my 2 heads over all 4096 tokens
            with ExitStack() as c2:
                p2 = c2.enter_context(tc.tile_pool(name="p2", bufs=1))
                p2ps = c2.enter_context(tc.tile_pool(name="p2ps", bufs=1, space="PSUM"))
                qwT_sb = p2.tile([128, NH * HPC * D], F32R)
                nc.sync.dma_start(
                    qwT_sb[:].rearrange("p (k m) -> p k m", k=NH),
                    qwT_in.rearrange("(k p) m -> p k m", p=128))
                for g in range(4):            # 1024-token groups
                    b, half = g // 2, g % 2
                    psq = [[p2ps.tile([128, 512], F32, name=f"psq{g}{m}{t2}",
                                      tag="psq", bufs=8)
                            for t2 in range(2)] for m in range(HPC)]
                    for k in range(NH):
                        ht = p2.tile([128, 1024], F32R, tag="hsq", bufs=4)
                        nc.sync.dma_start(
                            ht[:], hsT_in[k * 128:(k + 1) * 128,
                                          g * 1024:(g + 1) * 1024])
                        for m in range(HPC):
                            for t2 in range(2):
                                nc.tensor.matmul(
                                    psq[m][t2][:],
                                    qwT_sb[:, k * HPC * D + m * 128:
                                           k * HPC * D + (m + 1) * 128],
                                    ht[:, t2 * 512:(t2 + 1) * 512],
                                    start=(k == 0), stop=(k == NH - 1))
                    for m in range(HPC):
                        for t2 in range(2):
                            col = half * 1024 + t2 * 512
                            nc.any.tensor_copy(qT[m][b][:, col:col + 512],
                                               psq[m][t2][:])

            # ---------- P3: kv_b for my 2 heads over all tokens (+ v transpose)
            with ExitStack() as c3:
                p3 = c3.enter_context(tc.tile_pool(name="p3", bufs=1))
                p3ps = c3.enter_context(tc.tile_pool(name="p3ps", bufs=1, space="PSUM"))
                kvbT_sb = p3.tile([128, NR * HPC * 2 * D], F32R)
                nc.sync.dma_start(
                    kvbT_sb[:].rearrange("p (r m) -> p r m", r=NR),
                    kvbT_in.rearrange("(r p) m -> p r m", p=128))
                for tc8 in range(NCORES):     # 512-token chunks (AG layout)
                    b, loc = tc8 // 4, (tc8 % 4) * 512
                    lt = p3.tile([128, NR * 512], F32R, tag="lt", bufs=3)
                    nc.sync.dma_start(
                        lt[:].rearrange("p (r t) -> p r t", r=NR),
                        ag_out[tc8].rearrange("(r p) t -> p r t", p=128))
                    for m in range(2 * HPC):  # k0,v0,k1,v1
                        j, is_v = m // 2, m % 2
                        ps = p3ps.tile([128, 512], F32, tag="kv", bufs=4)
                        for r in range(NR):
                            nc.tensor.matmul(
                                ps[:],
                                kvbT_sb[:, r * HPC * 2 * D + m * 128:
                                        r * HPC * 2 * D + (m + 1) * 128],
                                lt[:, r * 512:(r + 1) * 512],
                                start=(r == 0), stop=(r == NR - 1))
                        if not is_v:
                            nc.any.tensor_copy(kT[j][b][:, loc:loc + 512], ps[:])
                        else:
                            vt = p3.tile([128, 512], F32R, tag="vt", bufs=2)
                            nc.any.tensor_copy(vt[:], ps[:])
                            for q4 in range(4):
                                pst = p3ps.tile([128, 128], F32R, tag="pst", bufs=2)
                                nc.tensor.transpose(
                                    pst[:], vt[:, q4 * 128:(q4 + 1) * 128], ident[:])
                                nc.any.tensor_copy(
                                    vnat[j][b][:, loc + q4 * 128: loc + (q4 + 1) * 128],
                                    pst[:])

            # ---------- P4: RoPE in place on qT and kT (rows 0:64)
            with ExitStack() as c4:
                p4 = c4.enter_context(tc.tile_pool(name="p4", bufs=1))
                for b in range(B):
                    for j in range(HPC):
                        for X in (qT[j][b], kT[j][b]):
                            tmp = p4.tile([ROPE, S], F32, tag="rtmp", bufs=2)
                            m2 = p4.tile([ROPE, S], F32, tag="rm2", bufs=2)
                            nc.vector.tensor_mul(tmp[:], X[0:ROPE], cosb[:])
                            nc.vector.tensor_mul(m2[0:32], X[32:64], sinsh[32:64])
                            nc.vector.tensor_mul(m2[32:64], X[0:32], sinsh[0:32])
                            nc.vector.tensor_add(X[0:ROPE], tmp[:], m2[:])

            # ---------- P5: attention per (batch, head), scoresT layout
            with ExitStack() as c5:
                p5 = c5.enter_context(tc.tile_pool(name="p5", bufs=1))
                p5ps = c5.enter_context(tc.tile_pool(name="p5ps", bufs=1, space="PSUM"))
                NKT = S // 128   # 16 k-chunks per batch
                for b in range(B):
                    for j in range(HPC):
                        qt, kt, vn, ao = qT[j][b], kT[j][b], vnat[j][b], aoT[j][b]
                        for qc in range(4):
                            qs = qt[:, qc * 512:(qc + 1) * 512]
                            ps_av = p5ps.tile([128, 512], F32, tag="av", bufs=2)
                            ps_den = p5ps.tile([1, 512], F32, tag="den", bufs=1)
                            for k16 in range(NKT):
                                ps_s = p5ps.tile([128, 512], F32, tag="s", bufs=3)
                                nc.tensor.matmul(
                                    ps_s[:], kt[:, k16 * 128:(k16 + 1) * 128], qs,
                                    start=True, stop=True)
                                e = p5.tile([128, 512], F32R, tag="e", bufs=6)
                                nc.scalar.activation(e[:], ps_s[:], AF.Exp)
                                nc.tensor.matmul(
                                    ps_av[:], vn[:, k16 * 128:(k16 + 1) * 128], e[:],
                                    start=(k16 == 0), stop=(k16 == NKT - 1))
                                nc.tensor.matmul(
                                    ps_den[:], ones[:, 0:1], e[:],
                                    start=(k16 == 0), stop=(k16 == NKT - 1))
                            den_sb = p5.tile([1, 512], F32R, tag="densb", bufs=2)
                            nc.vector.tensor_copy(den_sb[:], ps_den[:])
                            ps_bc = p5ps.tile([128, 512], F32, tag="bc", bufs=1)
                            nc.tensor.matmul(ps_bc[:], ones[0:1, :], den_sb[:],
                                             start=True, stop=True)
                            rec = p5.tile([128, 512], F32, tag="rec", bufs=2)
                            nc.vector.reciprocal(rec[:], ps_bc[:])
                            nc.vector.tensor_mul(ao[:, qc * 512:(qc + 1) * 512],
                                                 ps_av[:], rec[:])
                    # AllToAll for this batch as soon as both heads are done
                    for j in range(HPC):
                        for s8 in range(NCORES):
                            nc.sync.dma_start(
                                a2a_in[b][s8, j * D:(j + 1) * D, :],
                                aoT[j][b][:, s8 * SPC:(s8 + 1) * SPC])
                    nc.gpsimd.collective_compute(
                        "AllToAll", mybir.AluOpType.bypass, replica_groups=RG,
                        ins=[a2a_in[b].opt()], outs=[a2a_out[b].opt()])

        # ---------- P7: o_proj on my 512 tokens (256 per batch)
        with ExitStack() as c7:
            p7 = c7.enter_context(tc.tile_pool(name="p7", bufs=1))
            p7ps = c7.enter_context(tc.tile_pool(name="p7ps", bufs=1, space="PSUM"))
            af = p7.tile([128, NH * 2 * SPC], F32R)   # [128, 16*512]
            for k16 in range(NH):
                i, half = k16 // 2, k16 % 2
                for b in range(B):
                    nc.sync.dma_start(
                        af[:, k16 * 2 * SPC + b * SPC: k16 * 2 * SPC + (b + 1) * SPC],
                        a2a_out[b][i, half * 128:(half + 1) * 128, :])
            for om in range(NH):
                wt = p7.tile([128, HID], F32R, tag="ow", bufs=3)
                nc.sync.dma_start(wt[:], owt_in[om])
                ps_o = p7ps.tile([128, 2 * SPC], F32, tag="o", bufs=4)
                for k16 in range(NH):
                    nc.tensor.matmul(
                        ps_o[:], wt[:, k16 * 128:(k16 + 1) * 128],
                        af[:, k16 * 2 * SPC:(k16 + 1) * 2 * SPC],
                        start=(k16 == 0), stop=(k16 == NH - 1))
                o_sb = p7.tile([128, 2 * SPC], F32, tag="osb", bufs=3)
                nc.any.tensor_copy(o_sb[:], ps_o[:])
                nc.sync.dma_start(outT[om * 128:(om + 1) * 128, :], o_sb[:])

    nc.compile()
    return nc


def build_in_maps(hidden_states, q_w, kv_a_w, kv_b_w, o_w, kv_norm_w):
    hs = np.ascontiguousarray(np.asarray(hidden_states, dtype=np.float32))
    q_w = np.asarray(q_w, dtype=np.float32)
    kv_a_w = np.asarray(kv_a_w, dtype=np.float32)
    kv_b_w = np.asarray(kv_b_w, dtype=np.float32)
    o_w = np.asarray(o_w, dtype=np.float32)
    kv_norm_w = np.asarray(kv_norm_w, dtype=np.float32)

    hsT = tf32_round(np.ascontiguousarray(hs.reshape(T, HID).T))      # [HID, T]
    kvaT = tf32_round(np.ascontiguousarray(kv_a_w[ROPE:, :].T))       # [HID, RANK]
    scale = D ** -0.5
    cosT, sinT = _yarn_cos_sin()
    sinsh = np.concatenate([sinT[32:64], -sinT[0:32]], axis=0)
    ident = np.eye(128, dtype=np.float32)
    ones = np.ones((128, 128), dtype=np.float32)
    # owt[om, p, k*128+m] = o_w[om*128+m, k*128+p]
    owt = tf32_round(np.ascontiguousarray(
        o_w.reshape(16, 128, 16, 128).transpose(0, 3, 2, 1).reshape(16, 128, HID)))

    kvb = (kv_b_w * kv_norm_w[None, :]).reshape(H, 2, D, RANK)

    in_maps = []
    for c in range(NCORES):
        qwT = tf32_round(np.ascontiguousarray(
            (q_w[c * HPC * D:(c + 1) * HPC * D] * scale).T))           # [HID, 256]
        # kvbT rows order per core: k0,v0,k1,v1 each 128 wide
        blk = kvb[c * HPC:(c + 1) * HPC]                               # [2,2,128,RANK]
        kvbT = tf32_round(np.ascontiguousarray(
            blk.reshape(HPC * 2 * D, RANK).T))                         # [RANK, 512]
        hsmy = tf32_round(np.ascontiguousarray(
            hsT[:, c * TPC:(c + 1) * TPC]))
        in_maps.append({
            "hsT": hsT, "hsmy": hsmy, "qwT": qwT, "kvaT": kvaT,
            "kvbT": kvbT, "owt": owt, "cos": cosT, "sinsh": sinsh,
            "ident": ident, "ones": ones,
        })
    return in_maps


def assemble_output(results):
    out = np.empty((B, S, HID), dtype=np.float32)
    for c in range(NCORES):
        r = results[c]["outT"]                 # [HID, 2*SPC]
        out[0, c * SPC:(c + 1) * SPC, :] = r[:, 0:SPC].T
        out[1, c * SPC:(c + 1) * SPC, :] = r[:, SPC:2 * SPC].T
    return out


def kernel(hidden_states, q_w, kv_a_w, kv_b_w, o_w, kv_norm_w):
    from concourse import bass_utils

    if "nc" not in _CACHE:
        _CACHE["nc"] = build_nc()
    nc = _CACHE["nc"]
    in_maps = build_in_maps(hidden_states, q_w, kv_a_w, kv_b_w, o_w, kv_norm_w)
    res = bass_utils.run_bass_kernel_spmd(
        nc, in_maps, core_ids=list(range(NCORES)), trace=False)
    return assemble_output(res.results)
